# revision 50
# baseline (speedup 1.0000x reference)
"""APPNP (gnn_message_passing) distributed Trainium2 kernel, v3.

Algebra: the APPNP propagation is linear and W2 acts on features, so W2
commutes with propagation: we propagate y = relu(x@W1+b1)@W2 (one scalar
per node) instead of 64-wide h. Further, the GCN edge weight is separable,
w_e = dinv[src]*dinv[dst], so we propagate z = dinv*y:
    z_{k+1} = A (.) gathersum(z_k) + B (.) z_k + C,   A = 0.9*dinv^2,
    B = 0.9*wself, C = a*z_0, out = z_K / dinv + b2,
where gathersum[dst] = sum over non-self in-edges of z[src]; the edge
mask becomes an EXACT {0,1,2} table, resident in SBUF as fp8.

Layout per NeuronCore: nodes relabeled by (shard, in-degree); 8 Q7-core
groups of 16 partition lanes each; z replicated per lane-block so each
ap_gather pop yields the 16 candidate blocks at one offset. Columns are
slot-major per chunk: chunk = rank range [r0,r1) with uniform padded
degree maxd; column c0 + i*NR + (r-r0) = slot i of rank r. The segment
sum over slots runs on the PE: per slot-level i one matmul with the
block-ones stationary, accumulating into PSUM [8, NR]. Chunks are
bin-packed into ap_gather instruction groups >= BLK columns wide
(instruction cost scales with max AP free-size, so narrow gathers are
overcharged) with a deliberately tiny final group to shorten the
serial per-step tail; group starts stay 32-column aligned (the gather
idx slice base must be 4-byte aligned). The per-step serial chain is
AllGather (emitted with an unoptimized 2D output AP) -> z replication
-> gathers -> mask-mul -> PE segment-sum -> epilogue -> next AllGather.
"""

import os
import numpy as np

N = 100000
E = 1600000
D_IN = 256
D_H = 64
K = 10
ALPHA = 0.1
NCORES = 8
P = 128
PADN = 12544          # padded nodes per shard (8 groups x 1568)
NGRP = 8              # Q7-core groups per NeuronCore
GRPR = PADN // NGRP   # 1568 dst ranks per group
SHARD = N // NCORES   # 12500 real nodes per shard
DEVN = NCORES * PADN  # 100352 global device ids
BLK = DEVN // 16      # 6272: y block per partition lane
GCHUNKS = 12


def _preprocess(edge_index):
    row = np.asarray(edge_index[0], dtype=np.int64)
    col = np.asarray(edge_index[1], dtype=np.int64)
    loop = np.arange(N, dtype=np.int64)
    rows = np.concatenate([row, loop])
    cols = np.concatenate([col, loop])
    deg = np.bincount(cols, minlength=N).astype(np.int64)
    dinv = 1.0 / np.sqrt(deg.astype(np.float64))

    # Relabel: ascending in-degree, dealt round-robin to shards, then within
    # each shard round-robin to the 8 Q7-core groups -> every (shard, group)
    # has a nearly identical degree profile at each rank.
    order = np.argsort(deg, kind="stable")
    rank = np.empty(N, dtype=np.int64)
    rank[order] = np.arange(N)
    shard_of = (rank % NCORES).astype(np.int32)
    rho2 = rank // NCORES                    # [0, 12500) within shard
    grp_of = (rho2 % NGRP).astype(np.int32)  # Q7 core group
    rr = rho2 // NGRP
    counts = np.zeros((NCORES, NGRP), dtype=np.int64)
    for s in range(NCORES):
        counts[s] = np.bincount(grp_of[shard_of == s], minlength=NGRP)
    maxcnt = counts.max()
    assert maxcnt <= GRPR
    r_of = (rr + (GRPR - maxcnt)).astype(np.int64)   # same offset everywhere
    flat_of = grp_of.astype(np.int64) * GRPR + r_of  # [0, 12544)
    dev_of = shard_of.astype(np.int64) * PADN + flat_of

    # Per-node tables in [NGRP, GRPR] per-shard layout.
    # wselfraw[n] = sum over self-edges (incl. added loop) of dinv[n]^2
    selfmask = rows == cols
    nself = np.bincount(cols[selfmask], minlength=N).astype(np.float64)
    wselfraw = nself * dinv * dinv

    def to_table(vals):
        t = np.zeros((NCORES, NGRP, GRPR), dtype=np.float32)
        t[shard_of, grp_of, r_of] = vals.astype(np.float32)
        return t

    At = to_table((1.0 - ALPHA) * dinv * dinv)
    Bt = to_table((1.0 - ALPHA) * wselfraw)
    Dt = to_table(dinv)                       # z0 = dinv * y0
    St = to_table(1.0 / dinv)                 # y_K = z_K / dinv

    # gathered (non-self) edges: one column slot per unique (dst, o_src)
    nsr = rows[~selfmask]
    nsc = cols[~selfmask]
    o_all = (dev_of[nsr] % BLK).astype(np.int64)
    b_all = (dev_of[nsr] // BLK).astype(np.int64)
    pairkey = nsc * np.int64(BLK) + o_all
    # unique pairs with multiplicity (duplicate edges merge, mask += 1)
    upk, upk_inv, upk_cnt = np.unique(
        pairkey, return_inverse=True, return_counts=True)
    # for each unique pair: dst, o_src, and the set of source blocks.
    # Multiple blocks at same (dst,o) stay one column (different WT lanes).
    degg = np.bincount((upk // BLK).astype(np.int64), minlength=N)

    # per-rank gathered degree D[r] = max over (shard, group)
    D = np.zeros(GRPR, dtype=np.int64)
    np.maximum.at(D, r_of, degg)

    # DP chunking on 32-rank blocks: minimize sum(maxd * NR). 32 keeps
    # every chunk width (and so every gather-group start) a multiple of
    # 32 columns: the ap_gather idx slice base stays 4-byte aligned.
    NB = GRPR // 32
    Dmaxb = np.array([D[i * 32:(i + 1) * 32].max() for i in range(NB)])
    INF = float("inf")
    GC = GCHUNKS
    dp = np.full((NB + 1, GC + 1), INF)
    par = np.zeros((NB + 1, GC + 1), dtype=int)
    dp[0][0] = 0
    for j in range(1, NB + 1):
        mx = 0
        for i in range(j - 1, -1, -1):
            mx = max(mx, Dmaxb[i])
            for k in range(1, GC + 1):
                c = dp[i][k - 1] + mx * (j - i) * 32
                if c < dp[j][k]:
                    dp[j][k] = c
                    par[j][k] = i
    bounds = []
    j, k = NB, GC
    while k > 0:
        i = par[j][k]
        bounds.append((i * 32, j * 32))
        j, k = i, k - 1
    bounds.reverse()
    # raw chunks (r0, nr, maxd); column order assigned after bin-packing
    raw = []
    for (r0, r1) in bounds:
        nr = r1 - r0
        maxd = int(D[r0:r1].max())
        raw.append((int(r0), int(nr), maxd))
    tot = sum(nr * maxd for (_, nr, maxd) in raw)

    # Bin-pack chunks into gather groups balanced near tot/nbins, so each
    # ap_gather instruction is as wide as possible (>= BLK when feasible):
    # the modeled per-instruction cost is max(out_width, BLK) while the
    # real per-index cost only depends on total columns.
    # Reserve the smallest chunks (~2K cols) for a deliberately small final
    # bin: its mul/PE/epilogue tail after the last gather is the serial end
    # of each step, so keep it short. Remaining chunks are balanced over
    # nbins-1 bins, each as wide as possible (>= BLK keeps the modeled
    # per-instruction gather cost proportional to its real width).
    order = sorted(range(len(raw)), key=lambda i: raw[i][1] * raw[i][2])
    lastbin = []
    lw = 0
    for i in order:
        w = raw[i][1] * raw[i][2]
        if lw + w > 2300:
            break
        lastbin.append(i)
        lw += w
    rest = [i for i in order if i not in lastbin]
    nbins = max(1, int(round((tot - lw) / BLK)))
    bins = [[] for _ in range(nbins)]
    bw = [0] * nbins
    for i in sorted(rest, key=lambda i: -raw[i][1] * raw[i][2]):
        j = bw.index(min(bw))
        bins[j].append(i)
        bw[j] += raw[i][1] * raw[i][2]
    lastbin.sort(key=lambda i: -raw[i][1] * raw[i][2])
    bins.append(lastbin)
    # chunks in group order with assigned column offsets
    chunks = []
    ggroups = []
    c0 = 0
    for j in range(len(bins)):
        g0 = c0
        for i in bins[j]:
            r0, nr, maxd = raw[i]
            chunks.append((r0, nr, maxd, int(c0)))
            c0 += maxd * nr
        ggroups.append((int(g0), int(c0 - g0)))
    NI = int(c0)

    # per-rank chunk id and params
    chunk_of_rank = np.zeros(GRPR, dtype=np.int64)
    for ci, (r0, nr, maxd, cc0) in enumerate(chunks):
        chunk_of_rank[r0:r0 + nr] = ci
    c0_of_rank = np.array([chunks[chunk_of_rank[r]][3] for r in range(GRPR)])
    nr_of_rank = np.array([chunks[chunk_of_rank[r]][1] for r in range(GRPR)])
    r0_of_rank = np.array([chunks[chunk_of_rank[r]][0] for r in range(GRPR)])

    # slot index per unique pair within its dst (order within dst arbitrary)
    pdst_rank = rank[(upk // BLK).astype(np.int64)]       # global rank of dst
    po = (upk % BLK).astype(np.int64)                     # o_src
    sortk = np.argsort(pdst_rank, kind="stable")
    sp_rank = pdst_rank[sortk]
    pnew = np.empty(len(sp_rank), dtype=bool)
    pnew[0] = True
    pnew[1:] = sp_rank[1:] != sp_rank[:-1]
    prun = np.cumsum(pnew) - 1
    pfirst = np.full(prun[-1] + 1, len(sp_rank), dtype=np.int64)
    np.minimum.at(pfirst, prun, np.arange(len(sp_rank)))
    pslot_sorted = np.arange(len(sp_rank)) - pfirst[prun]
    pslot = np.empty(len(upk), dtype=np.int64)
    pslot[sortk] = pslot_sorted

    # column per unique pair: c = c0_chunk + slot * NR + (r_of[dst] - r0)
    u_dst = (upk // BLK).astype(np.int64)
    u_sh = shard_of[u_dst]
    u_gg = grp_of[u_dst]
    u_r = r_of[u_dst]
    u_c = c0_of_rank[u_r] + pslot * nr_of_rank[u_r] + (u_r - r0_of_rank[u_r])
    assert (pslot < np.array([chunks[chunk_of_rank[r]][2] for r in u_r])).all()

    # IDX[s][16*g + (c%16), c//16] = o_src
    IDX = np.zeros((NCORES, P, NI // 16), dtype=np.int16)
    IDX[u_sh, u_gg * 16 + (u_c % 16), u_c // 16] = po.astype(np.int16)

    # WT[s][16*g + b, c] += (# edges for that (pair, block))
    # loop over edges once (vectorized add at edge granularity)
    WT = np.zeros((NCORES, P, NI), dtype=np.float32)
    e_u = upk_inv                        # unique-pair id per edge
    np.add.at(WT,
              (u_sh[e_u], u_gg[e_u] * 16 + b_all, u_c[e_u]),
              1.0)

    return dict(shard_of=shard_of, flat_of=flat_of,
                IDX=IDX, WT=WT, NI=NI, chunks=chunks, ggroups=ggroups,
                At=At, Bt=Bt, Dt=Dt, St=St)


def _build_module(NI, chunks, ggroups):
    import concourse.bass as bass
    import concourse.bacc as bacc
    import concourse.mybir as mybir
    import concourse.tile as tile

    f32 = mybir.dt.float32
    bf16 = mybir.dt.bfloat16
    fp8 = mybir.dt.float8e4
    i16 = mybir.dt.int16

    nc = bacc.Bacc(None, target_bir_lowering=False, num_devices=NCORES)

    xT = nc.declare_dram_parameter("xT", [D_IN, PADN], bf16, isOutput=False)
    W1 = nc.declare_dram_parameter("W1", [D_IN, D_H], bf16, isOutput=False)
    b1 = nc.declare_dram_parameter("b1", [D_H, 1], f32, isOutput=False)
    W2 = nc.declare_dram_parameter("W2", [D_H, 1], bf16, isOutput=False)
    b2c = nc.declare_dram_parameter("b2c", [NGRP, 1], f32, isOutput=False)
    IDXp = nc.declare_dram_parameter("IDX", [P, NI // 16], i16, isOutput=False)
    WTp = nc.declare_dram_parameter("WT", [P, NI], fp8, isOutput=False)
    BOp = nc.declare_dram_parameter("BO", [P, NGRP], bf16, isOutput=False)
    Ap = nc.declare_dram_parameter("A", [NGRP, GRPR], f32, isOutput=False)
    Bp = nc.declare_dram_parameter("B", [NGRP, GRPR], bf16, isOutput=False)
    Dp = nc.declare_dram_parameter("Dv", [NGRP, GRPR], f32, isOutput=False)
    Sp = nc.declare_dram_parameter("S", [NGRP, GRPR], f32, isOutput=False)
    out = nc.declare_dram_parameter("out", [PADN], f32, isOutput=True)

    y0d = nc.dram_tensor("y0d", [1, PADN], f32, kind="Internal")
    agouts = [
        nc.dram_tensor(f"agout{i}", [NCORES, PADN], f32,
                       kind="Internal", addr_space="Shared")
        for i in range(2)
    ]
    agins = [
        nc.dram_tensor(f"agin{i}", [1, PADN], f32, kind="Internal")
        for i in range(K)
    ]

    CH = 512
    n_full, rem = divmod(PADN, CH)

    with tile.TileContext(nc) as tc:
        with (
            tc.tile_pool(name="const", bufs=1) as constp,
            tc.tile_pool(name="xtp", bufs=2) as xtp,
            tc.tile_pool(name="h0p", bufs=2) as h0p,
            tc.tile_pool(name="psum1", bufs=2, space="PSUM") as psum1p,
            tc.tile_pool(name="psum2", bufs=2, space="PSUM") as psum2p,
            tc.tile_pool(name="psumA", bufs=3, space="PSUM") as psumAp,
            tc.tile_pool(name="yrp", bufs=1) as yrp,
            tc.tile_pool(name="gp", bufs=3) as gp,
            tc.tile_pool(name="sgp", bufs=1) as sgp,
            tc.tile_pool(name="mp", bufs=2) as mp,
            tc.tile_pool(name="ep", bufs=1) as epp,
        ):
            w1sb = constp.tile([128, 2 * D_H], bf16, tag="w1")
            nc.sync.dma_start(w1sb[:, 0:D_H], W1[0:128, :])
            nc.sync.dma_start(w1sb[:, D_H:2 * D_H], W1[128:256, :])
            w2sb = constp.tile([D_H, 1], bf16, tag="w2")
            nc.sync.dma_start(w2sb[:], W2[:])
            b1sb = constp.tile([D_H, 1], f32, tag="b1")
            nc.sync.dma_start(b1sb[:], b1[:])
            b2sb = constp.tile([NGRP, 1], f32, tag="b2")
            nc.sync.dma_start(b2sb[:], b2c[:])
            idxsb = constp.tile([P, NI // 16], i16, tag="idx")
            nc.sync.dma_start(idxsb[:], IDXp[:])
            wtsb = constp.tile([P, NI], fp8, tag="wt")
            nc.sync.dma_start(wtsb[:], WTp[:])
            bosb = constp.tile([P, NGRP], bf16, tag="bo")
            nc.sync.dma_start(bosb[:], BOp[:])
            Asb = constp.tile([NGRP, GRPR], f32, tag="A")
            nc.sync.dma_start(Asb[:], Ap[:])
            Bsb = constp.tile([NGRP, GRPR], bf16, tag="B")
            nc.sync.dma_start(Bsb[:], Bp[:])

            # ---- stage A: y0 = relu(x @ W1 + b1) @ W2 ----
            QW = PADN // 4  # 3136 cols per xT quarter-load
            for q in range(4):
                xt0 = xtp.tile([128, QW], bf16, tag="xt")
                xt1 = xtp.tile([128, QW], bf16, tag="xt")
                nc.sync.dma_start(xt0[:], xT[0:128, q * QW:(q + 1) * QW])
                nc.sync.dma_start(xt1[:], xT[128:256, q * QW:(q + 1) * QW])
                for ci in range(QW // CH + (1 if QW % CH else 0)):
                    c0l = ci * CH
                    cn = min(CH, QW - c0l)
                    c0 = q * QW + c0l
                    ps1 = psum1p.tile([D_H, cn], f32, tag="ps1")
                    nc.tensor.matmul(ps1[:], w1sb[:, 0:D_H],
                                     xt0[:, c0l:c0l + cn],
                                     start=True, stop=False)
                    nc.tensor.matmul(ps1[:], w1sb[:, D_H:2 * D_H],
                                     xt1[:, c0l:c0l + cn],
                                     start=False, stop=True)
                    h0t = h0p.tile([D_H, cn], bf16, tag="h0t")
                    nc.scalar.activation(h0t[:], ps1[:],
                                         mybir.ActivationFunctionType.Relu,
                                         bias=b1sb[:])
                    ps2 = psum2p.tile([1, cn], f32, tag="ps2")
                    nc.tensor.matmul(ps2[:], w2sb[:], h0t[:],
                                     start=True, stop=True)
                    y0c = h0p.tile([1, cn], f32, tag="y0c")
                    nc.vector.tensor_copy(y0c[:], ps2[:])
                    nc.sync.dma_start(y0d[0, c0:c0 + cn], y0c[:])
            # z0 = dinv * y0 ; C = alpha * z0
            y0s = constp.tile([NGRP, GRPR], f32, tag="y0s")
            nc.sync.dma_start(
                y0s[:], y0d[0, :].rearrange("(g r) -> g r", g=NGRP))
            zA = constp.tile([NGRP, GRPR], f32, tag="zA")
            zB = constp.tile([NGRP, GRPR], f32, tag="zB")
            Csb = constp.tile([NGRP, GRPR], bf16, tag="C")
            nc.sync.dma_start(zB[:], Dp[:])
            nc.vector.tensor_mul(zA[:], zB[:], y0s[:])
            nc.vector.tensor_scalar_mul(Csb[:], zA[:], ALPHA)
            nc.sync.dma_start(
                agins[0][0, :].rearrange("(g r) -> g r", g=NGRP), zA[:])
            Ssb = y0s  # y0s is dead after Csb; reuse its space for S
            nc.sync.dma_start(Ssb[:], Sp[:])

            ztiles = [zA, zB]

            # ---- stage B: K propagation steps ----
            for k in range(K):
                zprev = ztiles[k % 2]
                znew = ztiles[(k + 1) % 2]
                agout = agouts[k % 2]
                # AllGather, emitted with an unoptimized (2D, contiguous)
                # output AP: [[PADN, 8], [1, PADN]].
                nc.has_collectives = True
                nc.gpsimd.add_instruction(
                    mybir.InstCollectiveCompute(
                        name=f"I-{nc.next_id()}",
                        kind="AllGather",
                        op=mybir.AluOpType.bypass,
                        replica_groups=[list(range(NCORES))],
                        ins=[nc.gpsimd.lower_ap(agins[k][:])],
                        outs=[nc.gpsimd.lower_ap(agout[:, :], opt=False)],
                        unique_tensors="No",
                        cc_dim="Partition",
                    ))
                # z_rep[16c+b, :] = z block b (8 group replicas)
                yrep = yrp.tile([P, BLK], f32, tag="yrep")
                yview = agout[:].rearrange("a b -> (a b)").rearrange(
                    "(b e) -> b e", b=16)
                for c in range(NGRP):
                    nc.sync.dma_start(yrep[16 * c:16 * c + 16, :], yview)

                # B*zprev + C depends only on the previous z: compute it
                # once per step, early, overlapped with the gathers.
                ubc = epp.tile([NGRP, GRPR], bf16, tag="ubc")
                nc.vector.tensor_mul(ubc[:], zprev[:], Bsb[:])
                nc.vector.tensor_add(ubc[:], ubc[:], Csb[:])

                gtiles = {}
                for (gc0, gcw) in ggroups:
                    pool = sgp if gcw < 4096 else gp
                    g = pool.tile([P, gcw], f32, tag="g")
                    nc.gpsimd.ap_gather(
                        out_ap=g[:].rearrange("p (i d) -> p i d", d=1),
                        in_ap=yrep[:].rearrange("p (e d) -> p e d", d=1),
                        idxs_ap=idxsb[:, gc0 // 16:(gc0 + gcw) // 16],
                        channels=P, num_elems=BLK, d=1, num_idxs=gcw,
                    )
                    gtiles[gc0] = (g, gc0, gcw)

                for (r0, nr, maxd, c0) in chunks:
                    cw = maxd * nr
                    for (g, gc0, gcw) in gtiles.values():
                        if gc0 <= c0 < gc0 + gcw:
                            break
                    off = c0 - gc0
                    m = mp.tile([P, cw], bf16, tag="m")
                    nc.vector.tensor_mul(m[:], g[:, off:off + cw],
                                         wtsb[:, c0:c0 + cw])
                    ps = psumAp.tile([NGRP, nr], f32, tag="psA")
                    for i in range(maxd):
                        nc.tensor.matmul(ps[:], bosb[:],
                                         m[:, i * nr:(i + 1) * nr],
                                         start=(i == 0), stop=(i == maxd - 1))
                    # per-chunk: znew_slice = A * gathersum
                    sl = slice(r0, r0 + nr)
                    nc.vector.tensor_mul(znew[:, sl], ps[:], Asb[:, sl])
                # combine and ship the new z in one go
                nc.vector.tensor_add(znew[:], znew[:], ubc[:])
                if k + 1 < K:
                    nc.sync.dma_start(
                        agins[k + 1][0, :].rearrange("(g r) -> g r", g=NGRP),
                        znew[:])
                else:
                    nc.vector.tensor_mul(zprev[:], znew[:], Ssb[:])
                    nc.vector.tensor_scalar_add(zprev[:], zprev[:], b2sb[:])
                    nc.sync.dma_start(
                        out[:].rearrange("(g r) -> g r", g=NGRP), zprev[:])

    nc.compile()
    return nc


_CACHE = {}


def _install_profile_hook():
    import sys
    import types
    try:
        from antenv import axon_hooks  # noqa: F401
        return True
    except ImportError:
        pass
    try:
        from trn_agent_boot.trn_boot import _ntff_profile_via_ctypes
        hook = _ntff_profile_via_ctypes("/opt/axon/libaxon_pjrt.so")
        if hook is None:
            return False
        mod = types.ModuleType("antenv.axon_hooks")
        mod._hook = hook
        mod.get_axon_ntff_profile_hook = lambda: mod._hook
        mod.set_axon_ntff_profile_hook = lambda h: setattr(mod, "_hook", h)
        sys.modules["antenv.axon_hooks"] = mod
        import antenv
        antenv.axon_hooks = mod
        return True
    except Exception:
        return False


def kernel(x, edge_index, W1, b1, W2, b2):
    import ml_dtypes
    from concourse.bass_utils import run_bass_kernel_spmd

    x = np.asarray(x)
    edge_index = np.asarray(edge_index)
    W1 = np.asarray(W1, dtype=np.float32)
    b1 = np.asarray(b1, dtype=np.float32)
    W2 = np.asarray(W2, dtype=np.float32)
    b2 = np.asarray(b2, dtype=np.float32)

    ekey = edge_index.tobytes()[:4096] + str(edge_index.sum()).encode()
    if "prep" in _CACHE and _CACHE.get("ekey") == ekey:
        prep = _CACHE["prep"]
        nc = _CACHE["nc"]
    else:
        prep = _preprocess(edge_index)
        nc = _build_module(prep["NI"], prep["chunks"], prep["ggroups"])
        _CACHE.update(prep=prep, nc=nc, ekey=ekey)

    shard_of = prep["shard_of"]
    flat_of = prep["flat_of"]

    bf16 = ml_dtypes.bfloat16
    xTs = np.zeros((NCORES, D_IN, PADN), dtype=bf16)
    xf = np.ascontiguousarray(x.astype(np.float32).T)
    for s in range(NCORES):
        m = shard_of == s
        xTs[s][:, flat_of[m]] = xf[:, m].astype(bf16)
    W1b = W1.astype(bf16)
    W2b = W2.reshape(D_H, 1).astype(bf16)
    b1c = b1.reshape(D_H, 1).astype(np.float32)
    b2c = np.full((NGRP, 1), float(b2.reshape(-1)[0]), dtype=np.float32)
    BO = np.zeros((P, NGRP), dtype=bf16)
    for c in range(NGRP):
        BO[16 * c:16 * c + 16, c] = 1.0

    in_maps = []
    for s in range(NCORES):
        in_maps.append({
            "xT": xTs[s], "W1": W1b, "b1": b1c, "W2": W2b, "b2c": b2c,
            "IDX": prep["IDX"][s],
            "WT": prep["WT"][s].astype(ml_dtypes.float8_e4m3),
            "BO": BO,
            "A": prep["At"][s],
            "B": prep["Bt"][s].astype(bf16),
            "Dv": prep["Dt"][s], "S": prep["St"][s],
        })

    trace = bool(os.environ.get("BASS_PROFILE")) and _install_profile_hook()
    res = run_bass_kernel_spmd(
        nc, in_maps, core_ids=list(range(NCORES)), trace=trace)
    _CACHE["last_result"] = res

    outs = res.results
    full = np.empty((N,), dtype=np.float32)
    for s in range(NCORES):
        m = shard_of == s
        full[m] = np.asarray(outs[s]["out"], dtype=np.float32)[flat_of[m]]
    return full.reshape(N, 1)


# revision 54
# speedup vs baseline: 1.0069x; 1.0069x over previous
"""APPNP (gnn_message_passing) distributed Trainium2 kernel, v3.

Algebra: the APPNP propagation is linear and W2 acts on features, so W2
commutes with propagation: we propagate y = relu(x@W1+b1)@W2 (one scalar
per node) instead of 64-wide h. Further, the GCN edge weight is separable,
w_e = dinv[src]*dinv[dst], so we propagate z = dinv*y:
    z_{k+1} = A (.) gathersum(z_k) + B (.) z_k + C,   A = 0.9*dinv^2,
    B = 0.9*wself, C = a*z_0, out = z_K / dinv + b2,
where gathersum[dst] = sum over non-self in-edges of z[src]; the edge
mask becomes an EXACT {0,1,2} table, resident in SBUF as fp8.

Layout per NeuronCore: nodes relabeled by (shard, in-degree); 8 Q7-core
groups of 16 partition lanes each; z replicated per lane-block so each
ap_gather pop yields the 16 candidate blocks at one offset. Columns are
slot-major per chunk: chunk = rank range [r0,r1) with uniform padded
degree maxd; column c0 + i*NR + (r-r0) = slot i of rank r. The segment
sum over slots runs on the PE: per slot-level i one matmul with the
block-ones stationary, accumulating into PSUM [8, NR]. Chunks are
bin-packed into ap_gather instruction groups >= BLK columns wide
(instruction cost scales with max AP free-size, so narrow gathers are
overcharged) with a deliberately tiny final group to shorten the
serial per-step tail; group starts stay 32-column aligned (the gather
idx slice base must be 4-byte aligned). The per-step serial chain is
AllGather (emitted with an unoptimized 2D output AP) -> z replication
-> gathers -> mask-mul -> PE segment-sum -> epilogue -> next AllGather.
"""

import os
import numpy as np

N = 100000
E = 1600000
D_IN = 256
D_H = 64
K = 10
ALPHA = 0.1
NCORES = 8
P = 128
PADN = 12544          # padded nodes per shard (8 groups x 1568)
NGRP = 8              # Q7-core groups per NeuronCore
GRPR = PADN // NGRP   # 1568 dst ranks per group
SHARD = N // NCORES   # 12500 real nodes per shard
DEVN = NCORES * PADN  # 100352 global device ids
BLK = DEVN // 16      # 6272: y block per partition lane
GCHUNKS = 12


def _preprocess(edge_index):
    row = np.asarray(edge_index[0], dtype=np.int64)
    col = np.asarray(edge_index[1], dtype=np.int64)
    loop = np.arange(N, dtype=np.int64)
    rows = np.concatenate([row, loop])
    cols = np.concatenate([col, loop])
    deg = np.bincount(cols, minlength=N).astype(np.int64)
    dinv = 1.0 / np.sqrt(deg.astype(np.float64))

    # Relabel: ascending in-degree, dealt round-robin to shards, then within
    # each shard round-robin to the 8 Q7-core groups -> every (shard, group)
    # has a nearly identical degree profile at each rank.
    order = np.argsort(deg, kind="stable")
    rank = np.empty(N, dtype=np.int64)
    rank[order] = np.arange(N)
    shard_of = (rank % NCORES).astype(np.int32)
    rho2 = rank // NCORES                    # [0, 12500) within shard
    grp_of = (rho2 % NGRP).astype(np.int32)  # Q7 core group
    rr = rho2 // NGRP
    counts = np.zeros((NCORES, NGRP), dtype=np.int64)
    for s in range(NCORES):
        counts[s] = np.bincount(grp_of[shard_of == s], minlength=NGRP)
    maxcnt = counts.max()
    assert maxcnt <= GRPR
    r_of = (rr + (GRPR - maxcnt)).astype(np.int64)   # same offset everywhere
    flat_of = grp_of.astype(np.int64) * GRPR + r_of  # [0, 12544)
    dev_of = shard_of.astype(np.int64) * PADN + flat_of

    # Per-node tables in [NGRP, GRPR] per-shard layout.
    # wselfraw[n] = sum over self-edges (incl. added loop) of dinv[n]^2
    selfmask = rows == cols
    nself = np.bincount(cols[selfmask], minlength=N).astype(np.float64)
    wselfraw = nself * dinv * dinv

    def to_table(vals):
        t = np.zeros((NCORES, NGRP, GRPR), dtype=np.float32)
        t[shard_of, grp_of, r_of] = vals.astype(np.float32)
        return t

    At = to_table((1.0 - ALPHA) * dinv * dinv)
    Bt = to_table((1.0 - ALPHA) * wselfraw)
    Dt = to_table(dinv)                       # z0 = dinv * y0
    St = to_table(1.0 / dinv)                 # y_K = z_K / dinv

    # gathered (non-self) edges: one column slot per unique (dst, o_src)
    nsr = rows[~selfmask]
    nsc = cols[~selfmask]
    o_all = (dev_of[nsr] % BLK).astype(np.int64)
    b_all = (dev_of[nsr] // BLK).astype(np.int64)
    pairkey = nsc * np.int64(BLK) + o_all
    # unique pairs with multiplicity (duplicate edges merge, mask += 1)
    upk, upk_inv, upk_cnt = np.unique(
        pairkey, return_inverse=True, return_counts=True)
    # for each unique pair: dst, o_src, and the set of source blocks.
    # Multiple blocks at same (dst,o) stay one column (different WT lanes).
    degg = np.bincount((upk // BLK).astype(np.int64), minlength=N)

    # per-rank gathered degree D[r] = max over (shard, group)
    D = np.zeros(GRPR, dtype=np.int64)
    np.maximum.at(D, r_of, degg)

    # DP chunking on 32-rank blocks: minimize sum(maxd * NR). 32 keeps
    # every chunk width (and so every gather-group start) a multiple of
    # 32 columns: the ap_gather idx slice base stays 4-byte aligned.
    NB = GRPR // 32
    Dmaxb = np.array([D[i * 32:(i + 1) * 32].max() for i in range(NB)])
    INF = float("inf")
    GC = GCHUNKS
    dp = np.full((NB + 1, GC + 1), INF)
    par = np.zeros((NB + 1, GC + 1), dtype=int)
    dp[0][0] = 0
    for j in range(1, NB + 1):
        mx = 0
        for i in range(j - 1, -1, -1):
            mx = max(mx, Dmaxb[i])
            for k in range(1, GC + 1):
                c = dp[i][k - 1] + mx * (j - i) * 32
                if c < dp[j][k]:
                    dp[j][k] = c
                    par[j][k] = i
    bounds = []
    j, k = NB, GC
    while k > 0:
        i = par[j][k]
        bounds.append((i * 32, j * 32))
        j, k = i, k - 1
    bounds.reverse()
    # raw chunks (r0, nr, maxd); column order assigned after bin-packing
    raw = []
    for (r0, r1) in bounds:
        nr = r1 - r0
        maxd = int(D[r0:r1].max())
        raw.append((int(r0), int(nr), maxd))
    tot = sum(nr * maxd for (_, nr, maxd) in raw)

    # Bin-pack chunks into gather groups balanced near tot/nbins, so each
    # ap_gather instruction is as wide as possible (>= BLK when feasible):
    # the modeled per-instruction cost is max(out_width, BLK) while the
    # real per-index cost only depends on total columns.
    # Reserve the smallest chunks (~2K cols) for a deliberately small final
    # bin: its mul/PE/epilogue tail after the last gather is the serial end
    # of each step, so keep it short. Remaining chunks are balanced over
    # nbins-1 bins, each as wide as possible (>= BLK keeps the modeled
    # per-instruction gather cost proportional to its real width).
    order = sorted(range(len(raw)), key=lambda i: raw[i][1] * raw[i][2])
    lastbin = []
    lw = 0
    for i in order:
        w = raw[i][1] * raw[i][2]
        if lw + w > 2300:
            break
        lastbin.append(i)
        lw += w
    rest = [i for i in order if i not in lastbin]
    nbins = max(1, int(round((tot - lw) / BLK)))
    bins = [[] for _ in range(nbins)]
    bw = [0] * nbins
    for i in sorted(rest, key=lambda i: -raw[i][1] * raw[i][2]):
        j = bw.index(min(bw))
        bins[j].append(i)
        bw[j] += raw[i][1] * raw[i][2]
    lastbin.sort(key=lambda i: -raw[i][1] * raw[i][2])
    bins.append(lastbin)
    # chunks in group order with assigned column offsets
    chunks = []
    ggroups = []
    c0 = 0
    for j in range(len(bins)):
        g0 = c0
        for i in bins[j]:
            r0, nr, maxd = raw[i]
            chunks.append((r0, nr, maxd, int(c0)))
            c0 += maxd * nr
        ggroups.append((int(g0), int(c0 - g0)))
    NI = int(c0)

    # per-rank chunk id and params
    chunk_of_rank = np.zeros(GRPR, dtype=np.int64)
    for ci, (r0, nr, maxd, cc0) in enumerate(chunks):
        chunk_of_rank[r0:r0 + nr] = ci
    c0_of_rank = np.array([chunks[chunk_of_rank[r]][3] for r in range(GRPR)])
    nr_of_rank = np.array([chunks[chunk_of_rank[r]][1] for r in range(GRPR)])
    r0_of_rank = np.array([chunks[chunk_of_rank[r]][0] for r in range(GRPR)])

    # slot index per unique pair within its dst (order within dst arbitrary)
    pdst_rank = rank[(upk // BLK).astype(np.int64)]       # global rank of dst
    po = (upk % BLK).astype(np.int64)                     # o_src
    sortk = np.argsort(pdst_rank, kind="stable")
    sp_rank = pdst_rank[sortk]
    pnew = np.empty(len(sp_rank), dtype=bool)
    pnew[0] = True
    pnew[1:] = sp_rank[1:] != sp_rank[:-1]
    prun = np.cumsum(pnew) - 1
    pfirst = np.full(prun[-1] + 1, len(sp_rank), dtype=np.int64)
    np.minimum.at(pfirst, prun, np.arange(len(sp_rank)))
    pslot_sorted = np.arange(len(sp_rank)) - pfirst[prun]
    pslot = np.empty(len(upk), dtype=np.int64)
    pslot[sortk] = pslot_sorted

    # column per unique pair: c = c0_chunk + slot * NR + (r_of[dst] - r0)
    u_dst = (upk // BLK).astype(np.int64)
    u_sh = shard_of[u_dst]
    u_gg = grp_of[u_dst]
    u_r = r_of[u_dst]
    u_c = c0_of_rank[u_r] + pslot * nr_of_rank[u_r] + (u_r - r0_of_rank[u_r])
    assert (pslot < np.array([chunks[chunk_of_rank[r]][2] for r in u_r])).all()

    # IDX[s][16*g + (c%16), c//16] = o_src
    IDX = np.zeros((NCORES, P, NI // 16), dtype=np.int16)
    IDX[u_sh, u_gg * 16 + (u_c % 16), u_c // 16] = po.astype(np.int16)

    # WT[s][16*g + b, c] += (# edges for that (pair, block))
    # loop over edges once (vectorized add at edge granularity)
    WT = np.zeros((NCORES, P, NI), dtype=np.float32)
    e_u = upk_inv                        # unique-pair id per edge
    np.add.at(WT,
              (u_sh[e_u], u_gg[e_u] * 16 + b_all, u_c[e_u]),
              1.0)

    return dict(shard_of=shard_of, flat_of=flat_of,
                IDX=IDX, WT=WT, NI=NI, chunks=chunks, ggroups=ggroups,
                At=At, Bt=Bt, Dt=Dt, St=St)


def _build_module(NI, chunks, ggroups):
    import concourse.bass as bass
    import concourse.bacc as bacc
    import concourse.mybir as mybir
    import concourse.tile as tile

    f32 = mybir.dt.float32
    bf16 = mybir.dt.bfloat16
    fp8 = mybir.dt.float8e4
    i16 = mybir.dt.int16

    nc = bacc.Bacc(None, target_bir_lowering=False, num_devices=NCORES)

    xT = nc.declare_dram_parameter("xT", [D_IN, PADN], bf16, isOutput=False)
    W1 = nc.declare_dram_parameter("W1", [D_IN, D_H], bf16, isOutput=False)
    b1 = nc.declare_dram_parameter("b1", [D_H, 1], f32, isOutput=False)
    W2 = nc.declare_dram_parameter("W2", [D_H, 1], bf16, isOutput=False)
    b2c = nc.declare_dram_parameter("b2c", [NGRP, 1], f32, isOutput=False)
    IDXp = nc.declare_dram_parameter("IDX", [P, NI // 16], i16, isOutput=False)
    WTp = nc.declare_dram_parameter("WT", [P, NI], fp8, isOutput=False)
    BOp = nc.declare_dram_parameter("BO", [P, NGRP], bf16, isOutput=False)
    Ap = nc.declare_dram_parameter("A", [NGRP, GRPR], f32, isOutput=False)
    Bp = nc.declare_dram_parameter("B", [NGRP, GRPR], bf16, isOutput=False)
    Dp = nc.declare_dram_parameter("Dv", [NGRP, GRPR], f32, isOutput=False)
    Sp = nc.declare_dram_parameter("S", [NGRP, GRPR], f32, isOutput=False)
    out = nc.declare_dram_parameter("out", [PADN], f32, isOutput=True)

    y0d = nc.dram_tensor("y0d", [1, PADN], f32, kind="Internal")
    agouts = [
        nc.dram_tensor(f"agout{i}", [NCORES, PADN], f32,
                       kind="Internal", addr_space="Shared")
        for i in range(2)
    ]
    agins = [
        nc.dram_tensor(f"agin{i}", [1, PADN], f32, kind="Internal")
        for i in range(K)
    ]

    CH = 512
    n_full, rem = divmod(PADN, CH)

    with tile.TileContext(nc) as tc:
        with (
            tc.tile_pool(name="const", bufs=1) as constp,
            tc.tile_pool(name="xtp", bufs=2) as xtp,
            tc.tile_pool(name="h0p", bufs=2) as h0p,
            tc.tile_pool(name="psum1", bufs=2, space="PSUM") as psum1p,
            tc.tile_pool(name="psum2", bufs=2, space="PSUM") as psum2p,
            tc.tile_pool(name="psumA", bufs=3, space="PSUM") as psumAp,
            tc.tile_pool(name="yrp", bufs=1) as yrp,
            tc.tile_pool(name="gp", bufs=3) as gp,
            tc.tile_pool(name="sgp", bufs=1) as sgp,
            tc.tile_pool(name="mp", bufs=2) as mp,
            tc.tile_pool(name="ep", bufs=1) as epp,
        ):
            w1sb = constp.tile([128, 2 * D_H], bf16, tag="w1")
            nc.sync.dma_start(w1sb[:, 0:D_H], W1[0:128, :])
            nc.sync.dma_start(w1sb[:, D_H:2 * D_H], W1[128:256, :])
            w2sb = constp.tile([D_H, 1], bf16, tag="w2")
            nc.sync.dma_start(w2sb[:], W2[:])
            b1sb = constp.tile([D_H, 1], f32, tag="b1")
            nc.sync.dma_start(b1sb[:], b1[:])
            b2sb = constp.tile([NGRP, 1], f32, tag="b2")
            nc.sync.dma_start(b2sb[:], b2c[:])
            idxsb = constp.tile([P, NI // 16], i16, tag="idx")
            nc.sync.dma_start(idxsb[:], IDXp[:])
            wtsb = constp.tile([P, NI], fp8, tag="wt")
            nc.sync.dma_start(wtsb[:], WTp[:])
            bosb = constp.tile([P, NGRP], bf16, tag="bo")
            nc.sync.dma_start(bosb[:], BOp[:])
            Asb = constp.tile([NGRP, GRPR], f32, tag="A")
            nc.sync.dma_start(Asb[:], Ap[:])
            Bsb = constp.tile([NGRP, GRPR], bf16, tag="B")
            nc.sync.dma_start(Bsb[:], Bp[:])

            # ---- stage A: y0 = relu(x @ W1 + b1) @ W2 ----
            QW = PADN // 4  # 3136 cols per xT quarter-load
            for q in range(4):
                xt0 = xtp.tile([128, QW], bf16, tag="xt")
                xt1 = xtp.tile([128, QW], bf16, tag="xt")
                nc.sync.dma_start(xt0[:], xT[0:128, q * QW:(q + 1) * QW])
                nc.sync.dma_start(xt1[:], xT[128:256, q * QW:(q + 1) * QW])
                for ci in range(QW // CH + (1 if QW % CH else 0)):
                    c0l = ci * CH
                    cn = min(CH, QW - c0l)
                    c0 = q * QW + c0l
                    ps1 = psum1p.tile([D_H, cn], f32, tag="ps1")
                    nc.tensor.matmul(ps1[:], w1sb[:, 0:D_H],
                                     xt0[:, c0l:c0l + cn],
                                     start=True, stop=False)
                    nc.tensor.matmul(ps1[:], w1sb[:, D_H:2 * D_H],
                                     xt1[:, c0l:c0l + cn],
                                     start=False, stop=True)
                    h0t = h0p.tile([D_H, cn], bf16, tag="h0t")
                    nc.scalar.activation(h0t[:], ps1[:],
                                         mybir.ActivationFunctionType.Relu,
                                         bias=b1sb[:])
                    ps2 = psum2p.tile([1, cn], f32, tag="ps2")
                    nc.tensor.matmul(ps2[:], w2sb[:], h0t[:],
                                     start=True, stop=True)
                    y0c = h0p.tile([1, cn], f32, tag="y0c")
                    nc.scalar.activation(y0c[:], ps2[:],
                                         mybir.ActivationFunctionType.Copy)
                    nc.sync.dma_start(y0d[0, c0:c0 + cn], y0c[:])
            # z0 = dinv * y0 ; C = alpha * z0
            y0s = constp.tile([NGRP, GRPR], f32, tag="y0s")
            nc.sync.dma_start(
                y0s[:], y0d[0, :].rearrange("(g r) -> g r", g=NGRP))
            zA = constp.tile([NGRP, GRPR], f32, tag="zA")
            zB = constp.tile([NGRP, GRPR], f32, tag="zB")
            Csb = constp.tile([NGRP, GRPR], bf16, tag="C")
            nc.sync.dma_start(zB[:], Dp[:])
            nc.vector.tensor_mul(zA[:], zB[:], y0s[:])
            nc.vector.tensor_scalar_mul(Csb[:], zA[:], ALPHA)
            nc.sync.dma_start(
                agins[0][0, :].rearrange("(g r) -> g r", g=NGRP), zA[:])
            Ssb = y0s  # y0s is dead after Csb; reuse its space for S
            nc.sync.dma_start(Ssb[:], Sp[:])

            ztiles = [zA, zB]

            # ---- stage B: K propagation steps ----
            for k in range(K):
                zprev = ztiles[k % 2]
                znew = ztiles[(k + 1) % 2]
                agout = agouts[k % 2]
                # AllGather, emitted with an unoptimized (2D, contiguous)
                # output AP: [[PADN, 8], [1, PADN]].
                nc.has_collectives = True
                nc.gpsimd.add_instruction(
                    mybir.InstCollectiveCompute(
                        name=f"I-{nc.next_id()}",
                        kind="AllGather",
                        op=mybir.AluOpType.bypass,
                        replica_groups=[list(range(NCORES))],
                        ins=[nc.gpsimd.lower_ap(agins[k][:])],
                        outs=[nc.gpsimd.lower_ap(
                            agout[:].rearrange("a b -> (a b)").rearrange(
                                "(x y) -> x y", y=GRPR), opt=False)],
                        unique_tensors="No",
                        cc_dim="Partition",
                    ))
                # z_rep[16c+b, :] = z block b (8 group replicas)
                yrep = yrp.tile([P, BLK], f32, tag="yrep")
                yview = agout[:].rearrange("a b -> (a b)").rearrange(
                    "(b e) -> b e", b=16)
                for c in range(NGRP):
                    nc.sync.dma_start(yrep[16 * c:16 * c + 16, :], yview)

                # B*zprev + C depends only on the previous z: compute it
                # once per step, early, overlapped with the gathers.
                ubc = epp.tile([NGRP, GRPR], bf16, tag="ubc")
                nc.vector.tensor_mul(ubc[:], zprev[:], Bsb[:])
                nc.vector.tensor_add(ubc[:], ubc[:], Csb[:])

                gtiles = {}
                for (gc0, gcw) in ggroups:
                    pool = sgp if gcw < 4096 else gp
                    g = pool.tile([P, gcw], f32, tag="g")
                    nc.gpsimd.ap_gather(
                        out_ap=g[:].rearrange("p (i d) -> p i d", d=1),
                        in_ap=yrep[:].rearrange("p (e d) -> p e d", d=1),
                        idxs_ap=idxsb[:, gc0 // 16:(gc0 + gcw) // 16],
                        channels=P, num_elems=BLK, d=1, num_idxs=gcw,
                    )
                    gtiles[gc0] = (g, gc0, gcw)

                for (r0, nr, maxd, c0) in chunks:
                    cw = maxd * nr
                    for (g, gc0, gcw) in gtiles.values():
                        if gc0 <= c0 < gc0 + gcw:
                            break
                    off = c0 - gc0
                    m = mp.tile([P, cw], bf16, tag="m")
                    nc.vector.tensor_mul(m[:], g[:, off:off + cw],
                                         wtsb[:, c0:c0 + cw])
                    ps = psumAp.tile([NGRP, nr], f32, tag="psA")
                    for i in range(maxd):
                        nc.tensor.matmul(ps[:], bosb[:],
                                         m[:, i * nr:(i + 1) * nr],
                                         start=(i == 0), stop=(i == maxd - 1))
                    # per-chunk: znew_slice = A * gathersum
                    sl = slice(r0, r0 + nr)
                    nc.vector.tensor_mul(znew[:, sl], ps[:], Asb[:, sl])
                # combine and ship the new z in one go
                nc.vector.tensor_add(znew[:], znew[:], ubc[:])
                if k + 1 < K:
                    nc.sync.dma_start(
                        agins[k + 1][0, :].rearrange("(g r) -> g r", g=NGRP),
                        znew[:])
                else:
                    nc.vector.tensor_mul(zprev[:], znew[:], Ssb[:])
                    nc.vector.tensor_scalar_add(zprev[:], zprev[:], b2sb[:])
                    nc.sync.dma_start(
                        out[:].rearrange("(g r) -> g r", g=NGRP), zprev[:])

    nc.compile()
    return nc


_CACHE = {}


def _install_profile_hook():
    import sys
    import types
    try:
        from antenv import axon_hooks  # noqa: F401
        return True
    except ImportError:
        pass
    try:
        from trn_agent_boot.trn_boot import _ntff_profile_via_ctypes
        hook = _ntff_profile_via_ctypes("/opt/axon/libaxon_pjrt.so")
        if hook is None:
            return False
        mod = types.ModuleType("antenv.axon_hooks")
        mod._hook = hook
        mod.get_axon_ntff_profile_hook = lambda: mod._hook
        mod.set_axon_ntff_profile_hook = lambda h: setattr(mod, "_hook", h)
        sys.modules["antenv.axon_hooks"] = mod
        import antenv
        antenv.axon_hooks = mod
        return True
    except Exception:
        return False


def kernel(x, edge_index, W1, b1, W2, b2):
    import ml_dtypes
    from concourse.bass_utils import run_bass_kernel_spmd

    x = np.asarray(x)
    edge_index = np.asarray(edge_index)
    W1 = np.asarray(W1, dtype=np.float32)
    b1 = np.asarray(b1, dtype=np.float32)
    W2 = np.asarray(W2, dtype=np.float32)
    b2 = np.asarray(b2, dtype=np.float32)

    ekey = edge_index.tobytes()[:4096] + str(edge_index.sum()).encode()
    if "prep" in _CACHE and _CACHE.get("ekey") == ekey:
        prep = _CACHE["prep"]
        nc = _CACHE["nc"]
    else:
        prep = _preprocess(edge_index)
        nc = _build_module(prep["NI"], prep["chunks"], prep["ggroups"])
        _CACHE.update(prep=prep, nc=nc, ekey=ekey)

    shard_of = prep["shard_of"]
    flat_of = prep["flat_of"]

    bf16 = ml_dtypes.bfloat16
    xTs = np.zeros((NCORES, D_IN, PADN), dtype=bf16)
    xf = np.ascontiguousarray(x.astype(np.float32).T)
    for s in range(NCORES):
        m = shard_of == s
        xTs[s][:, flat_of[m]] = xf[:, m].astype(bf16)
    W1b = W1.astype(bf16)
    W2b = W2.reshape(D_H, 1).astype(bf16)
    b1c = b1.reshape(D_H, 1).astype(np.float32)
    b2c = np.full((NGRP, 1), float(b2.reshape(-1)[0]), dtype=np.float32)
    BO = np.zeros((P, NGRP), dtype=bf16)
    for c in range(NGRP):
        BO[16 * c:16 * c + 16, c] = 1.0

    in_maps = []
    for s in range(NCORES):
        in_maps.append({
            "xT": xTs[s], "W1": W1b, "b1": b1c, "W2": W2b, "b2c": b2c,
            "IDX": prep["IDX"][s],
            "WT": prep["WT"][s].astype(ml_dtypes.float8_e4m3),
            "BO": BO,
            "A": prep["At"][s],
            "B": prep["Bt"][s].astype(bf16),
            "Dv": prep["Dt"][s], "S": prep["St"][s],
        })

    trace = bool(os.environ.get("BASS_PROFILE")) and _install_profile_hook()
    res = run_bass_kernel_spmd(
        nc, in_maps, core_ids=list(range(NCORES)), trace=trace)
    _CACHE["last_result"] = res

    outs = res.results
    full = np.empty((N,), dtype=np.float32)
    for s in range(NCORES):
        m = shard_of == s
        full[m] = np.asarray(outs[s]["out"], dtype=np.float32)[flat_of[m]]
    return full.reshape(N, 1)
